# revision 8
# baseline (speedup 1.0000x reference)
"""Trainium2 Bass kernel for the MACE-style symmetric contraction.

Math (per node b, feature c, with emb = node_embeddings[b, c, :] (16,)):
    w{3,2,1}[k, c] = sum_e attr[b, e] * W{3,2,1}[e, k, c]
    out3[x, y] = sum_{i,k} emb[i] * w3[k] * U3[0, x, y, i, k]        (16, 16)
    M3[x, y]   = out3[x, y] + sum_k2 U2[0, x, y, k2] * w2[k2]
    o2[x]      = sum_y M3[x, y] * emb[y] + U1[0, x, 0] * w1[0]
    o1         = sum_x o2[x] * emb[x]
    output[b, c] = o1

Mapping: columns = (node-in-tile, c) pairs, 4 nodes x 128 c = 512 cols/tile.
The (i, k) contraction (k-major, 368 rows + 4 U2 rows) runs on the PE as
3 accumulating matmuls per output half (x,y) -> 256 rows in 2 halves of 128.
The y- and x-contractions with emb are elementwise multiplies (DVE) plus
selection/ones matmuls (PE). All PE operands are f16; accumulation is fp32.
"""

import os

import numpy as np

# ---------------- problem constants (hardcoded per contract) ----------------
N, C, Y, E = 3000, 128, 16, 10
Z3, Z2, Z1 = 23, 4, 1
NCORES = 8
NB = 376                # nodes per core (3008 = 8*376, padded)
NPAD = NCORES * NB
TB = 4                  # nodes per tile
F = TB * C              # 512 columns per tile
NT = NB // TB           # 94 tiles
KK = Z3 + Z2 + Z1       # 28 packed k rows in wflat
WROW = KK * C           # 3584: wflat row length
KM = (128, 128, 116)    # contraction chunk K sizes (368 ik rows + 4 U2 rows)

_CACHE = {}


def _build_program(nb):
    """Build the single-core Bass program (SPMD: same program, all cores)."""
    import concourse.bass as bass
    import concourse.mybir as mybir
    import concourse.tile as tile
    from concourse import bacc

    f16, f32 = mybir.dt.float16, mybir.dt.float32
    nt = nb // TB
    nc = bacc.Bacc(None, target_bir_lowering=False)

    embT_d = nc.dram_tensor("embT", [Y, nb * C], f16, kind="ExternalInput")
    attrT_d = nc.dram_tensor("attrT", [E, nb], f16, kind="ExternalInput")
    wcat_d = nc.dram_tensor("wcat", [E, WROW], f16, kind="ExternalInput")
    u3s_d = nc.dram_tensor("u3s", [2, 3, 128, 128], f16, kind="ExternalInput")
    sel_d = nc.dram_tensor("sel", [2, 128, 16], f16, kind="ExternalInput")
    u1row_d = nc.dram_tensor("u1row", [1, 16], f16, kind="ExternalInput")
    ones_d = nc.dram_tensor("ones16", [16, 1], f16, kind="ExternalInput")
    out_d = nc.dram_tensor("out", [nb, C], f32, kind="ExternalOutput")

    with tile.TileContext(nc) as tc:
        with tc.tile_pool(name="consts", bufs=1) as consts, \
             tc.tile_pool(name="dram", bufs=1, space="DRAM") as dpool:
            # stationaries, loaded once
            u3s = []
            for h in range(2):
                row = []
                for m in range(3):
                    t = consts.tile([128, 128], f16, tag=f"u3s{h}{m}")
                    nc.sync.dma_start(out=t[:], in_=u3s_d[h, m])
                    row.append(t)
                u3s.append(row)
            sel = []
            for h in range(2):
                t = consts.tile([128, 16], f16, tag=f"sel{h}")
                nc.sync.dma_start(out=t[:], in_=sel_d[h])
                sel.append(t)
            u1row = consts.tile([1, 16], f16, tag="u1row")
            nc.sync.dma_start(out=u1row[:], in_=u1row_d[:])
            ones16 = consts.tile([16, 1], f16, tag="ones16")
            nc.sync.dma_start(out=ones16[:], in_=ones_d[:])

            # wflatT[kk, node*C + c] = sum_e attr[node, e] * Wcat[e, kk*C + c]
            nbC = nb * C
            wflatT = dpool.tile([KK, nbC], f16, tag="wflatT")

            # ---------------- phase A: produce wflatT ----------------
            with tc.tile_pool(name="pa", bufs=2) as pa, \
                 tc.tile_pool(name="psA", bufs=2, space="PSUM") as psA:
                attrT = pa.tile([E, nb], f16, tag="attrT")
                nc.sync.dma_start(out=attrT[:], in_=attrT_d[:])
                wcat = pa.tile([E, WROW], f16, tag="wcat")
                nc.sync.dma_start(out=wcat[:], in_=wcat_d[:])
                wflatT_ap = wflatT[:]
                for gs in range(0, nb, 128):
                    gn = min(128, nb - gs)
                    for j in range(WROW // 512):
                        pw = psA.tile([128, 512], f32, tag="pw")
                        nc.tensor.matmul(
                            pw[:gn],
                            lhsT=attrT[:, gs:gs + gn],
                            rhs=wcat[:, 512 * j:512 * (j + 1)],
                            start=True, stop=True,
                        )
                        wf = pa.tile([128, 512], f16, tag="wf")
                        nc.scalar.copy(wf[:gn], pw[:gn])
                        # scatter-transpose: (node, 4 kk, c) -> wflatT rows
                        # SWDGE (gpsimd): HWDGE queue descriptors allow only
                        # one sync wait and this DMA needs two.
                        nc.gpsimd.dma_start(
                            out=bass.AP(
                                tensor=wflatT_ap.tensor,
                                offset=wflatT_ap.offset + 4 * j * nbC + gs * C,
                                ap=[[C, gn], [nbC, 4], [1, C]],
                            ),
                            in_=wf[:gn],
                        )

            # ---------------- phase B: main loop ----------------
            tc.strict_bb_all_engine_barrier()
            wflatT_ap = wflatT[:]
            embT_ap = embT_d[:]

            def wflat_gather(kk0, col0, kcnt, irep):
                """AP over wflatT: rows (k, i-rep), cols = F contiguous."""
                ap = [[nbC, kcnt]]
                if irep > 1:
                    ap.append([0, irep])
                ap += [[1, F]]
                return bass.AP(
                    tensor=wflatT_ap.tensor,
                    offset=wflatT_ap.offset + kk0 * nbC + col0,
                    ap=ap,
                )

            with tc.tile_pool(name="st", bufs=4) as st, \
                 tc.tile_pool(name="pP", bufs=4, space="PSUM") as pP, \
                 tc.tile_pool(name="pP1", bufs=2, space="PSUM") as pP1, \
                 tc.tile_pool(name="pP2", bufs=2, space="PSUM") as pP2:
                for t in range(nt):
                    node0 = TB * t
                    col0 = node0 * C
                    # --- loads ---
                    embT = st.tile([Y, F], f16, tag="embT")
                    nc.sync.dma_start(out=embT[:], in_=embT_d[:, col0:col0 + F])
                    embB = st.tile([128, F], f16, tag="embB")
                    nc.sync.dma_start(
                        out=embB[:],
                        in_=bass.AP(
                            tensor=embT_ap.tensor,
                            offset=embT_ap.offset + col0,
                            ap=[[0, 8], [nb * C, Y], [1, F]],
                        ),
                    )
                    wm0 = st.tile([128, F], f16, tag="wm0")
                    nc.sync.dma_start(out=wm0[:], in_=wflat_gather(0, col0, 8, Y))
                    wm1 = st.tile([128, F], f16, tag="wm1")
                    nc.sync.dma_start(out=wm1[:], in_=wflat_gather(8, col0, 8, Y))
                    wm2 = st.tile([112, F], f16, tag="wm2")
                    nc.sync.dma_start(out=wm2[:], in_=wflat_gather(16, col0, 7, Y))
                    w1row = st.tile([1, F], f16, tag="w1row")
                    nc.sync.dma_start(out=w1row[:], in_=wflat_gather(27, col0, 1, 1))

                    # --- G = emb_i * w3_k (moving operand), U2 rows appended ---
                    g0 = st.tile([128, F], f16, tag="g0")
                    nc.gpsimd.tensor_mul(g0[:], embB[:], wm0[:])
                    g1 = st.tile([128, F], f16, tag="g1")
                    nc.gpsimd.tensor_mul(g1[:], embB[:], wm1[:])
                    g2 = st.tile([116, F], f16, tag="g2")
                    nc.sync.dma_start(
                        out=g2[112:116], in_=wflat_gather(23, col0, 4, 1)
                    )
                    nc.vector.tensor_mul(g2[:112], embB[:112], wm2[:])
                    gs_ = (g0, g1, g2)

                    # --- main contraction: out3[(x,y) half, col] ---
                    P = []
                    for h in range(2):
                        ph = pP.tile([128, F], f32, tag="P")
                        for m in range(3):
                            nc.tensor.matmul(
                                ph[:],
                                lhsT=u3s[h][m][:KM[m]],
                                rhs=gs_[m][:KM[m]],
                                start=(m == 0), stop=(m == 2),
                            )
                        P.append(ph)

                    # --- S = M3 * emb_y ; y-reduction + U1 outer product ---
                    p1 = pP1.tile([16, F], f32, tag="P1")
                    S = []
                    for h in range(2):
                        sh = st.tile([128, F], f16, tag=f"s{h}")
                        nc.vector.tensor_mul(sh[:], P[h][:], embB[:])
                        S.append(sh)
                    nc.tensor.matmul(p1[:], lhsT=sel[0][:], rhs=S[0][:],
                                     start=True, stop=False)
                    nc.tensor.matmul(p1[:], lhsT=sel[1][:], rhs=S[1][:],
                                     start=False, stop=False)
                    nc.tensor.matmul(p1[:], lhsT=u1row[:], rhs=w1row[:],
                                     start=False, stop=True)

                    # --- S2 = o2 * emb_x ; x-reduction ---
                    s2 = st.tile([16, F], f16, tag="s2")
                    nc.vector.tensor_mul(s2[:], p1[:], embT[:])
                    p2 = pP2.tile([1, F], f32, tag="P2")
                    nc.tensor.matmul(p2[:], lhsT=ones16[:], rhs=s2[:],
                                     start=True, stop=True)

                    # --- writeback ---
                    o1 = st.tile([1, F], f32, tag="o1")
                    nc.scalar.copy(o1[:], p2[:])
                    nc.sync.dma_start(out=out_d[node0:node0 + TB, :], in_=o1[:])
    nc.compile()
    return nc


# ---------------- host-side input preparation ----------------

def _prep_constants(U3, U2, U1):
    """Stationary operands: U3/U2 reordered to (k-major ik rows, (x,y) cols)."""
    U3 = np.asarray(U3, dtype=np.float32)
    U2 = np.asarray(U2, dtype=np.float32)
    U1 = np.asarray(U1, dtype=np.float32)
    # rows r=(k,i)=k*16+i, cols (x,y)=x*16+y
    U3r = U3[0].transpose(3, 2, 0, 1).reshape(Z3 * Y, Y * Y)
    U2r = U2[0].transpose(2, 0, 1).reshape(Z2, Y * Y)
    M = np.vstack([U3r, U2r])                       # (372, 256)
    u3s = np.zeros((2, 3, 128, 128), dtype=np.float16)
    for m in range(3):
        chunk = M[128 * m:128 * m + KM[m]]
        for h in range(2):
            u3s[h, m, :KM[m], :] = chunk[:, 128 * h:128 * (h + 1)]
    sel = np.zeros((2, 128, 16), dtype=np.float16)
    for h in range(2):
        for p in range(128):
            sel[h, p, 8 * h + p // 16] = 1.0
    u1row = U1[0, :, 0].reshape(1, Y).astype(np.float16)
    ones16 = np.ones((Y, 1), dtype=np.float16)
    return u3s, sel, u1row, ones16


def _prep_core_inputs(emb_pad, attr_pad, wcat, consts, g, nb=NB):
    u3s, sel, u1row, ones16 = consts
    sl = slice(g * nb, (g + 1) * nb)
    embT = np.ascontiguousarray(
        emb_pad[sl].transpose(2, 0, 1).reshape(Y, nb * C)
    ).astype(np.float16)
    attrT = np.ascontiguousarray(attr_pad[sl].T).astype(np.float16)
    return {
        "embT": embT,
        "attrT": attrT,
        "wcat": wcat,
        "u3s": u3s,
        "sel": sel,
        "u1row": u1row,
        "ones16": ones16,
    }


def _prep_all(node_embeddings, node_attributes, U3, U2, U1, W3, W2, W1):
    emb = np.asarray(node_embeddings, dtype=np.float32)
    attr = np.asarray(node_attributes, dtype=np.float32)
    emb_pad = np.zeros((NPAD, C, Y), dtype=np.float32)
    emb_pad[:N] = emb
    attr_pad = np.zeros((NPAD, E), dtype=np.float32)
    attr_pad[:N] = attr
    # wcat[e, kk*C + c]: kk 0..22 = W3, 23..26 = W2, 27 = W1
    wcat = np.concatenate(
        [np.asarray(W3, np.float32), np.asarray(W2, np.float32),
         np.asarray(W1, np.float32)], axis=1
    ).reshape(E, WROW).astype(np.float16)
    consts = _prep_constants(U3, U2, U1)
    return [
        _prep_core_inputs(emb_pad, attr_pad, wcat, consts, g)
        for g in range(NCORES)
    ]


def kernel(node_embeddings, node_attributes, U3, U2, U1, W3, W2, W1):
    from concourse.bass_utils import run_bass_kernel_spmd

    if "nc" not in _CACHE:
        _CACHE["nc"] = _build_program(NB)
    nc = _CACHE["nc"]
    in_maps = _prep_all(node_embeddings, node_attributes,
                        U3, U2, U1, W3, W2, W1)
    trace = bool(int(os.environ.get("KERNEL_TRACE", "0")))
    res = run_bass_kernel_spmd(
        nc, in_maps, core_ids=list(range(NCORES)), trace=trace,
    )
    _CACHE["last_results"] = res
    out = np.concatenate([res.results[g]["out"] for g in range(NCORES)], axis=0)
    return np.ascontiguousarray(out[:N]).astype(np.float32)
